# revision 46
# baseline (speedup 1.0000x reference)
"""Self-attention (CrossAttention module with q=k=v=x) kernel for Trainium2.

Problem: x [B=4, N=4096, H=256] fp32; Wq/Wk/Wv [256,256], bq/bk/bv [256].
  q = x@Wq.T+bq ; k = x@Wk.T+bk ; v = x@Wv.T+bv
  out = softmax(q@k.T) @ v          (no 1/sqrt(d) scaling)

Sharding: 8 cores = batch (4) x query-halves (2). Each core holds the full
K/V sequence for its batch element and 2048 query rows.

Scores algebra: q_i.k_j = x_i (Wq^T Wk) x_j^T + u_i + w_j + c where u_i, c
are constant per softmax row (dropped -- softmax-invariant) and
w_j = x_j . (Wk^T bq) is applied as the exp() bias on-device.  So the
device only needs qT = (x_half @ Wq^T Wk)^T, the raw keys xT = x^T, and
V = x @ Wv^T -- the small O(N H^2) projections are folded on the host
(f64), leaving the O(N^2 H) attention as pure device work.

Measured facts driving the schedule (from perfetto traces):
  - PE streams at the roofline once warm: N=512 scores matmul 216ns,
    N=257 AV matmul 110ns; total stream ~111.4us.  All wins are in the
    head (first data ~10-12us), the HAM clock-gate warmup, and the tail.
  - each dma_start costs ~0.65us of serial DIRECT2D descriptor-gen per
    HWDGE ring (sync/scalar; the rings start dispatching after a global
    barrier ~6.9us, first byte ~0.8-1.2us after doorbell).  SWDGE
    (gpsimd) is ~1.5us slower to first byte -- never on the head.
  - DMA-completion -> consumer-sem-observe latency is ~1.45us.
  - descriptor efficiency: 1KB+ rows ~180 GB/s/ring; 512B rows ~74 GB/s.
    So the critical head rides host-packed blobs, two chains per ring:
    hd chain 1 = [xT key chunk 0 | qT query block 0] (exactly the first
    matmul pair's operands, ~0.9us transfer), chain 2 = [xT keys
    128:512 | wsc bit-cast to fp16].  The separate thin xt/wsc chains
    these replace were the main DMA-jitter / sem-latency exposure.
  - HAM clock gate: PE runs at 1.2GHz until a full 3.4us activity window
    passes busy; a junk-matmul warmup bridges from ~7.5us seamlessly
    into the first scores matmul (any PE gap resets the window).
  - exp->AV latency (2 sem hops + ~290ns ACT op overhead) stalls the
    warm PE at each pair's first AV; pair 0 reorders jc1-b0's scores
    before AV-jc0 to cover it.
  - final accumulator split by V columns across two PSUM banks so the
    high half's normalize + DMA overlap the low half's AV sweep; the low
    half's normalize + out chains are split across both rings so their
    DIRECT2D dispatches run in parallel.  The tail floor is the out
    chain's HBM write-receipt sem (~1.4us) + teardown barrier (~2.5us).

fp8 was evaluated and rejected: DoubleRow (157 TF/s) needs BOTH operands
fp8; e4m3 scores inject ~0.17 logit noise (output err ~0.1 >> the 2e-2
gate) and e4m3 V has worst-case element error ~0.125 which a peaked
softmax row passes straight through.
"""

import sys

import numpy as np

if "/opt/trn_rl_repo" not in sys.path:
    sys.path.insert(0, "/opt/trn_rl_repo")

B, N, H = 4, 4096, 256
P = 128
NQ = N // 2          # query rows per core
JC = N // P          # key chunks (32)
IBLK = 512           # query block
ICH = IBLK // P      # query sub-chunks per block (4)
NPAIR = NQ // (2 * IBLK)  # block pairs per core (2)
HDX = 512            # xT columns packed into the head blob
HDW = HDX + IBLK + 64  # head blob width (+64 fp16 cols = wsc bit-cast)


def build_nc(salt=0, nwarm=80):
    import concourse.mybir as mybir
    import concourse.tile as tile
    from concourse import bacc

    f32 = mybir.dt.float32
    FR = mybir.dt.float16    # scores matmul dtype (11-bit mantissa)
    AVT = mybir.dt.bfloat16  # exp(S) and V dtype: needs fp32-like range
    Exp = mybir.ActivationFunctionType.Exp

    nc = bacc.Bacc("TRN2", target_bir_lowering=False, debug=False)

    xT_d = nc.dram_tensor("xT", [H, N], FR, kind="ExternalInput").ap()
    qT_d = nc.dram_tensor("qT", [H, NQ], FR, kind="ExternalInput").ap()
    hd_d = nc.dram_tensor("hd", [2, P, HDW], FR, kind="ExternalInput").ap()
    v_d = nc.dram_tensor("v", [N, H + 1], AVT, kind="ExternalInput").ap()
    att_d = nc.dram_tensor("att", [NQ, H], AVT, kind="ExternalOutput").ap()
    warm_d = nc.dram_tensor("warm", [P, 4], f32, kind="ExternalOutput").ap()

    with tile.TileContext(nc) as tc:
        with tc.tile_pool(name="io", bufs=1) as io, \
             tc.tile_pool(name="kqv", bufs=1) as kqv, \
             tc.tile_pool(name="expp", bufs=JC + 11) as expp, \
             tc.tile_pool(name="op", bufs=4) as op, \
             tc.tile_pool(name="psmm", bufs=3, space="PSUM") as psmm, \
             tc.tile_pool(name="psatt", bufs=5, space="PSUM") as psatt:

            xt = [io.tile([P, N], FR, tag=f"xt{h}", name=f"xt{h}") for h in range(2)]
            qt = [io.tile([P, NQ], FR, tag=f"qt{h}", name=f"qt{h}") for h in range(2)]
            hd = [io.tile([P, HDW], FR, tag=f"hd{h}", name=f"hd{h}") for h in range(2)]
            vt = [kqv.tile([P, H + 1], AVT, tag=f"v{j}", name=f"v{j}") for j in range(JC)]
            # per-key exp bias w_j rides the h=0 head blob as bit-cast
            # fp16 columns -- no separate (slow, 128B-row) wsc chain and
            # no extra sem before the first exp
            wsc = hd[0][:, HDX + IBLK:HDW].bitcast(f32)

            def ld_h(eng, sb_tiles, dr, h, cols):
                hs = slice(h * P, (h + 1) * P)
                eng.dma_start(sb_tiles[h][:, cols], dr[hs, cols])

            def ld_v(eng, j):
                eng.dma_start(vt[j][:], v_d[j * P:(j + 1) * P, :])

            # keys chunk stationary: the first 4 chunks live in the head
            # blob (their xt big-tile columns are never loaded).  Blob
            # layout: [xt jc0 | qt block0 | xt jc1..3 | wsc] so the first
            # chain (0:640) is exactly the first matmul pair's operands.
            def xstat(oc, jc):
                if jc == 0:
                    return hd[oc][:, 0:P]
                if jc < HDX // P:
                    return hd[oc][:, P + IBLK + (jc - 1) * P:
                                   P + IBLK + jc * P]
                return xt[oc][:, jc * P:(jc + 1) * P]

            # query block moving operand: block 0 lives in the head blob
            def qmov(oc, babs):
                if babs == 0:
                    return hd[oc][:, P:P + IBLK]
                return qt[oc][:, babs * IBLK:(babs + 1) * IBLK]

            # scalar ring: h=1 head blob (two chains: the first is just
            # the first matmul pair's operands, landing ~0.6us earlier)
            nc.scalar.dma_start(hd[1][:, 0:P + IBLK], hd_d[1, :, 0:P + IBLK])
            nc.scalar.dma_start(hd[1][:, P + IBLK:HDW], hd_d[1, :, P + IBLK:HDW])
            nc.scalar.dma_start(vt[1][:], v_d[P:2 * P, :])
            nc.scalar.dma_start(qt[1][:, 512:1024], qT_d[P:2 * P, 512:1024])

            # gpsimd SWDGE ring (otherwise idle): v tiles needed at
            # 17-22us; SWDGE's ~2.2us/chain serial emission delivers them
            # by ~16us, taking ~1.8us of load off the sync ring's
            # jitter-sensitive early window.
            for j in range(4, 8):
                ld_v(nc.gpsimd, j)

            # ---- PE warm-up on the const-AP region: it is memset during
            # the Bass preamble and fenced by the init all-engine barrier,
            # so the junk matmuls (and the dummy exp that pulls in the
            # ~2.7us ACT_TABLE_LOAD) start right after the barrier
            # (~7.0us) with no cross-engine sem hop.
            import concourse.mybir as _mybir
            cb = nc.const_aps.tensor(1.0, (P, P), _mybir.dt.bfloat16)
            cm = nc.const_aps.tensor(1.0, (P, 64), _mybir.dt.bfloat16)
            cw = nc.const_aps.tensor(0.0, (P, 2), f32)
            wex = op.tile([P, 2], f32, tag="wex", name="wex")
            nc.scalar.activation(wex[:], cw, Exp)

            # warmup matmuls: bridge until the head blob lands (~11.5us);
            # a PE gap before the first scores matmul would reset the HAM
            # activity window (costs ~2us of half-clock).
            wps = psmm.tile([P, 256], f32, tag="mm", name="wps")
            nw = nwarm + salt
            for r in range(nw):
                nc.tensor.matmul(wps[:, 0:64], cb, cm,
                                 start=(r == 0), stop=(r == nw - 1))
            wsb = op.tile([P, 4], f32, tag="wsb", name="wsb")
            nc.vector.tensor_copy(wsb[:, 0:2], wps[:, 0:2])
            nc.vector.tensor_copy(wsb[:, 2:4], wex[:])

            # sync ring: h=0 head blob, then wsc + v0 (small, they gate
            # the first exp/AV), then criticals and all bulk in
            # need-order (xt chains promoted ahead of same-time v tiles)
            nc.sync.dma_start(hd[0][:, 0:P + IBLK], hd_d[0, :, 0:P + IBLK])
            nc.sync.dma_start(hd[0][:, P + IBLK:HDW], hd_d[0, :, P + IBLK:HDW])
            ld_v(nc.sync, 0)
            ld_h(nc.sync, qt, qT_d, 0, slice(512, 1024))
            ld_h(nc.sync, xt, xT_d, 0, slice(512, 1024))
            ld_h(nc.sync, xt, xT_d, 1, slice(512, 1024))
            ld_v(nc.sync, 2)
            ld_v(nc.sync, 3)
            ld_h(nc.sync, xt, xT_d, 0, slice(1024, 2048))
            ld_h(nc.sync, xt, xT_d, 1, slice(1024, 2048))
            for j in range(8, 10):
                ld_v(nc.sync, j)
            ld_h(nc.sync, xt, xT_d, 0, slice(2048, 3072))
            ld_h(nc.sync, xt, xT_d, 1, slice(2048, 3072))
            for j in range(10, 18):
                ld_v(nc.sync, j)
            ld_h(nc.sync, xt, xT_d, 0, slice(3072, 4096))
            ld_h(nc.sync, xt, xT_d, 1, slice(3072, 4096))
            for j in range(18, 26):
                ld_v(nc.sync, j)
            ld_h(nc.sync, qt, qT_d, 0, slice(1024, 2048))
            ld_h(nc.sync, qt, qT_d, 1, slice(1024, 2048))
            for j in range(26, JC):
                ld_v(nc.sync, j)

            # warm-up flush (keeps junk matmuls + dummy exp alive through
            # DCE); last on the sync ring so it takes no early dispatch slot.
            nc.sync.dma_start(warm_d[:], wsb[:])

            # ---- attention block pairs ----
            # The scores matmuls for both blocks of a pair share each
            # key-chunk stationary load.  Block b0's AV runs inline per
            # key-chunk; block b1's exp(S) tiles are buffered in SBUF and
            # consumed in a second AV sweep (PSUM can only hold one
            # block's accumulators plus the rotating scores tiles).
            def normalize_one(att_tile, blk, ic):
                rec = op.tile([P, 1], f32, tag="rec", name="rec")
                nc.vector.reciprocal(rec[:], att_tile[:, H:H + 1])
                ao = op.tile([P, H], AVT, tag="ao", name="ao")
                nc.vector.tensor_scalar_mul(ao[:], att_tile[:, 0:H], rec[:])
                r0 = blk * IBLK + ic * P
                nc.sync.dma_start(att_d[r0:r0 + P, :], ao[:])

            def emit_scores(pair, jc, b, scs):
                babs = 2 * pair + b
                for oc in range(2):
                    nc.tensor.matmul(scs[:], xstat(oc, jc), qmov(oc, babs),
                                     start=(oc == 0), stop=(oc == 1))

            def emit_exp(jc, b, scs, split=False):
                ex = expp.tile([P, IBLK], AVT, tag="ex", name=f"ex{b}")
                if split:
                    # split the pair's first exp so its low half (feeding
                    # AV ic=0,1) is ready sooner (ACT has ~290ns/op fixed
                    # overhead, so only a 2-way split pays)
                    nc.scalar.activation(ex[:, 0:256], scs[:, 0:256], Exp,
                                         bias=wsc[:, jc:jc + 1])
                    nc.scalar.activation(ex[:, 256:IBLK], scs[:, 256:IBLK],
                                         Exp, bias=wsc[:, jc:jc + 1])
                else:
                    nc.scalar.activation(ex[:], scs[:], Exp,
                                         bias=wsc[:, jc:jc + 1])
                return ex

            def emit_av(att_ps, exs0, jc, vslice=None, ps_narrow=None):
                for ic in range(ICH):
                    ics = slice(ic * P, (ic + 1) * P)
                    nc.tensor.matmul(att_ps[ic][:], exs0[jc][:, ics],
                                     vt[jc][:],
                                     start=(jc == 0), stop=(jc == JC - 1))

            def emit_pair1_head():
                # pre-emit pair 1's jc0 (both blocks) + jc1-b0 scores and
                # exps; the PE runs them just before pair 0's final AV
                # sweep and the ACT ops complete during it, so pair 1's
                # first AV matmuls fire with no refill stall.
                ex0, ex1 = [], []
                s00 = psmm.tile([P, IBLK], f32, tag="mm", name="p1s00")
                emit_scores(1, 0, 0, s00)
                ex0.append(emit_exp(0, 0, s00, split=True))
                s01 = psmm.tile([P, IBLK], f32, tag="mm", name="p1s01")
                emit_scores(1, 0, 1, s01)
                ex1.append(emit_exp(0, 1, s01))
                s10 = psmm.tile([P, IBLK], f32, tag="mm", name="p1s10")
                emit_scores(1, 1, 0, s10)
                ex0.append(emit_exp(1, 0, s10))
                return ex0, ex1

            handoff = None
            for pair in range(NPAIR):
                att_ps = [psatt.tile([P, H + 1], f32, tag="att", name="attps")
                          for _ in range(ICH)]
                exs = [[], []]

                if pair == 0:
                    # data-arrival region: ALL of b0's first 4 key chunks
                    # (served by the head blob + early chains) run before
                    # any b1 work, giving the second-wave qt chains ~1.2us
                    # of jitter slack; AVs are interleaved so the exp->AV
                    # latency is covered by scores work.
                    sb0 = []
                    for jc in range(4):
                        s = psmm.tile([P, IBLK], f32, tag="mm",
                                      name=f"a{jc}")
                        emit_scores(0, jc, 0, s)
                        exs[0].append(emit_exp(jc, 0, s, split=(jc == 0)))
                        sb0.append(s)
                        if jc == 2:
                            emit_av(att_ps, exs[0], 0)
                        elif jc == 3:
                            emit_av(att_ps, exs[0], 1)
                    for jc in range(4):
                        s = psmm.tile([P, IBLK], f32, tag="mm",
                                      name=f"b{jc}")
                        emit_scores(0, jc, 1, s)
                        exs[1].append(emit_exp(jc, 1, s))
                        if jc == 1:
                            emit_av(att_ps, exs[0], 2)
                        elif jc == 2:
                            emit_av(att_ps, exs[0], 3)
                    jc_start = 4
                elif handoff is not None:
                    # pair 1's first scores+exps were pre-emitted into
                    # pair 0's final AV sweep (covers the pair-boundary
                    # exp->AV refill stall)
                    exs[0].append(handoff[0][0])
                    exs[0].append(handoff[0][1])
                    exs[1].append(handoff[1][0])
                    emit_av(att_ps, exs[0], 0)
                    sc11 = psmm.tile([P, IBLK], f32, tag="mm", name="sc11b")
                    emit_scores(1, 1, 1, sc11)
                    exs[1].append(emit_exp(1, 1, sc11))
                    emit_av(att_ps, exs[0], 1)
                    jc_start = 2
                else:
                    jc_start = 0

                for jc in range(jc_start, JC):
                    scs = [psmm.tile([P, IBLK], f32, tag="mm", name=f"sc{b}")
                           for b in range(2)]
                    if pair == 0 and jc < 4:
                        # block-major: b0's operands land ~1us before b1's
                        for b in range(2):
                            emit_scores(pair, jc, b, scs[b])
                            exs[b].append(emit_exp(jc, b, scs[b]))
                    else:
                        for oc in range(2):
                            for b in range(2):
                                nc.tensor.matmul(scs[b][:], xstat(oc, jc),
                                                 qmov(oc, 2 * pair + b),
                                                 start=(oc == 0),
                                                 stop=(oc == 1))
                        for b in range(2):
                            exs[b].append(emit_exp(jc, b, scs[b],
                                                   split=(jc == 0)))
                    emit_av(att_ps, exs[0], jc)

                for ic in range(ICH):
                    normalize_one(att_ps[ic], 2 * pair, ic)
                last = (pair == NPAIR - 1)
                for ic in range(ICH):
                    ics = slice(ic * P, (ic + 1) * P)
                    if last and ic == ICH - 1:
                        # final accumulator: split by V columns across two
                        # PSUM banks so the high half's normalize + DMA-out
                        # overlaps the low half's AV sweep.
                        pa = psatt.tile([P, H - P + 1], f32, tag="att",
                                        name="attpa")
                        pb = psatt.tile([P, P], f32, tag="att", name="attpb")
                        for jc in range(JC):
                            nc.tensor.matmul(pa[:], exs[1][jc][:, ics],
                                             vt[jc][:, P:H + 1],
                                             start=(jc == 0),
                                             stop=(jc == JC - 1))
                        rec = op.tile([P, 1], f32, tag="rec", name="rec")
                        nc.vector.reciprocal(rec[:], pa[:, H - P:H - P + 1])
                        ah = op.tile([P, H - P], AVT, tag="ao", name="ah")
                        nc.vector.tensor_scalar_mul(ah[:], pa[:, 0:H - P],
                                                    rec[:])
                        r0 = (2 * pair + 1) * IBLK + ic * P
                        nc.scalar.dma_start(att_d[r0:r0 + P, P:H], ah[:])
                        for jc in range(JC):
                            nc.tensor.matmul(pb[:], exs[1][jc][:, ics],
                                             vt[jc][:, 0:P],
                                             start=(jc == 0),
                                             stop=(jc == JC - 1))
                        # final normalize + out split in half across BOTH
                        # rings: the two DIRECT2D dispatches run in
                        # parallel and each 16KB transfer is ~0.5us vs
                        # ~0.9us for one 32KB chain at end-of-NEFF
                        al0 = op.tile([P, 64], AVT, tag="ao", name="al0")
                        nc.vector.tensor_scalar_mul(al0[:], pb[:, 0:64],
                                                    rec[:])
                        nc.scalar.dma_start(att_d[r0:r0 + P, 0:64], al0[:])
                        al1 = op.tile([P, 64], AVT, tag="ao", name="al1")
                        nc.vector.tensor_scalar_mul(al1[:], pb[:, 64:P],
                                                    rec[:])
                        nc.sync.dma_start(att_d[r0:r0 + P, 64:P], al1[:])
                    else:
                        if pair == 0 and ic == ICH - 1:
                            handoff = emit_pair1_head()
                        pf = psatt.tile([P, H + 1], f32, tag="att",
                                        name="attpsb")
                        for jc in range(JC):
                            nc.tensor.matmul(pf[:], exs[1][jc][:, ics],
                                             vt[jc][:],
                                             start=(jc == 0),
                                             stop=(jc == JC - 1))
                        normalize_one(pf, 2 * pair + 1, ic)

    nc.compile()
    return nc


_NC_CACHE = {}


def _get_nc():
    if "nc" not in _NC_CACHE:
        _NC_CACHE["nc"] = build_nc()
    return _NC_CACHE["nc"]


def _make_in_maps(x, Wq, bq, Wk, bk, Wv):
    import ml_dtypes

    bf16 = ml_dtypes.bfloat16
    A = Wq.T.astype(np.float64) @ Wk.astype(np.float64)
    wkbq = Wk.T.astype(np.float64) @ bq.astype(np.float64)
    in_maps = []
    for b in range(B):
        xb = x[b].astype(np.float64)
        wsc_b = np.ascontiguousarray(
            (xb @ wkbq).astype(np.float32).reshape(JC, P).T)
        v_b = np.empty((N, H + 1), dtype=bf16)
        v_b[:, 0:H] = (xb @ Wv.T.astype(np.float64)).astype(bf16)
        v_b[:, H:] = np.ones((N, 1), dtype=bf16)
        v_b = np.ascontiguousarray(v_b)
        xT_b = np.ascontiguousarray(x[b].astype(np.float16).T)
        q_b = (xb @ A).astype(np.float16)
        for half in range(2):
            qT = np.ascontiguousarray(q_b[half * NQ:(half + 1) * NQ, :].T)
            hd_b = np.empty((2, P, HDW), dtype=np.float16)
            for h in range(2):
                hs = slice(h * P, (h + 1) * P)
                hd_b[h, :, 0:P] = xT_b[hs, 0:P]
                hd_b[h, :, P:P + IBLK] = qT[hs, 0:IBLK]
                hd_b[h, :, P + IBLK:HDX + IBLK] = xT_b[hs, P:HDX]
                hd_b[h, :, HDX + IBLK:HDW] = wsc_b.view(np.float16)
            in_maps.append({"xT": xT_b, "qT": qT, "hd": hd_b, "v": v_b})
    return in_maps


def _run(inputs, trace=False):
    from concourse.bass_utils import run_bass_kernel_spmd

    x = np.asarray(inputs["x"], dtype=np.float32)
    Wq = np.asarray(inputs["Wq"], dtype=np.float32)
    bq = np.asarray(inputs["bq"], dtype=np.float32)
    Wk = np.asarray(inputs["Wk"], dtype=np.float32)
    bk = np.asarray(inputs["bk"], dtype=np.float32)
    Wv = np.asarray(inputs["Wv"], dtype=np.float32)
    bv = np.asarray(inputs["bv"], dtype=np.float32)

    in_maps = _make_in_maps(x, Wq, bq, Wk, bk, Wv)
    # The device occasionally wedges on the first execution of a fresh
    # NEFF (NRT_EXEC_UNIT_UNRECOVERABLE) or silently corrupts an output
    # (NaN/garbage); a retry -- with a slightly perturbed program
    # (different walrus schedule) on exception -- recovers.
    last_exc = None
    out = None
    for attempt in range(4):
        try:
            nc = _get_nc() if attempt < 2 else build_nc(salt=attempt)
            res = run_bass_kernel_spmd(nc, in_maps, list(range(8)), trace=trace)
        except Exception as e:  # noqa: BLE001
            last_exc = e
            import os as _os
            import time as _time
            _os.environ["NEURON_RT_RESET_CORES"] = "1"
            _time.sleep(3)
            continue
        out = np.empty((B, N, H), dtype=np.float32)
        for c in range(8):
            b, half = c // 2, c % 2
            out[b, half * NQ:(half + 1) * NQ, :] = \
                res.results[c]["att"].astype(np.float32) + bv
        if np.isfinite(out).all() and np.abs(out).max() < 1e3:
            return out, res
    if out is None:
        raise last_exc
    return out, res


def kernel(**inputs) -> np.ndarray:
    out, _ = _run(inputs, trace=False)
    return out


# revision 47
# speedup vs baseline: 1.0109x; 1.0109x over previous
"""Self-attention (CrossAttention module with q=k=v=x) kernel for Trainium2.

Problem: x [B=4, N=4096, H=256] fp32; Wq/Wk/Wv [256,256], bq/bk/bv [256].
  q = x@Wq.T+bq ; k = x@Wk.T+bk ; v = x@Wv.T+bv
  out = softmax(q@k.T) @ v          (no 1/sqrt(d) scaling)

Sharding: 8 cores = batch (4) x query-halves (2). Each core holds the full
K/V sequence for its batch element and 2048 query rows.

Scores algebra: q_i.k_j = x_i (Wq^T Wk) x_j^T + u_i + w_j + c where u_i, c
are constant per softmax row (dropped -- softmax-invariant) and
w_j = x_j . (Wk^T bq) is applied as the exp() bias on-device.  So the
device only needs qT = (x_half @ Wq^T Wk)^T, the raw keys xT = x^T, and
V = x @ Wv^T -- the small O(N H^2) projections are folded on the host
(f64), leaving the O(N^2 H) attention as pure device work.

Measured facts driving the schedule (from perfetto traces):
  - PE streams at the roofline once warm: N=512 scores matmul 216ns,
    N=257 AV matmul 110ns; total stream ~111.4us.  All wins are in the
    head (first data ~10-12us), the HAM clock-gate warmup, and the tail.
  - each dma_start costs ~0.65us of serial DIRECT2D descriptor-gen per
    HWDGE ring (sync/scalar; the rings start dispatching after a global
    barrier ~6.9us, first byte ~0.8-1.2us after doorbell).  SWDGE
    (gpsimd) is ~1.5us slower to first byte -- never on the head.
  - DMA-completion -> consumer-sem-observe latency is ~1.45us.
  - descriptor efficiency: 1KB+ rows ~180 GB/s/ring; 512B rows ~74 GB/s.
    So the critical head rides host-packed blobs, two chains per ring:
    hd chain 1 = [xT key chunk 0 | qT query block 0] (exactly the first
    matmul pair's operands, ~0.9us transfer), chain 2 = [xT keys
    128:512 | wsc bit-cast to fp16].  The separate thin xt/wsc chains
    these replace were the main DMA-jitter / sem-latency exposure.
  - HAM clock gate: PE runs at 1.2GHz until a full 3.4us activity window
    passes busy; a junk-matmul warmup bridges from ~7.5us seamlessly
    into the first scores matmul (any PE gap resets the window).
  - exp->AV latency (2 sem hops + ~290ns ACT op overhead) stalls the
    warm PE at each pair's first AV; pair 0 reorders jc1-b0's scores
    before AV-jc0 to cover it.
  - final accumulator split by V columns across two PSUM banks so the
    high half's normalize + DMA overlap the low half's AV sweep; the low
    half's normalize + out chains are split across both rings so their
    DIRECT2D dispatches run in parallel.  The tail floor is the out
    chain's HBM write-receipt sem (~1.4us) + teardown barrier (~2.5us).

fp8 was evaluated and rejected: DoubleRow (157 TF/s) needs BOTH operands
fp8; e4m3 scores inject ~0.17 logit noise (output err ~0.1 >> the 2e-2
gate) and e4m3 V has worst-case element error ~0.125 which a peaked
softmax row passes straight through.
"""

import sys

import numpy as np

if "/opt/trn_rl_repo" not in sys.path:
    sys.path.insert(0, "/opt/trn_rl_repo")

B, N, H = 4, 4096, 256
P = 128
NQ = N // 2          # query rows per core
JC = N // P          # key chunks (32)
IBLK = 512           # query block
ICH = IBLK // P      # query sub-chunks per block (4)
NPAIR = NQ // (2 * IBLK)  # block pairs per core (2)
HDX = 512            # xT columns packed into the head blob
HDW = HDX + IBLK + 64  # head blob width (+64 fp16 cols = wsc bit-cast)


def build_nc(salt=0, nwarm=80):
    import concourse.mybir as mybir
    import concourse.tile as tile
    from concourse import bacc

    f32 = mybir.dt.float32
    FR = mybir.dt.float16    # scores matmul dtype (11-bit mantissa)
    AVT = mybir.dt.bfloat16  # exp(S) and V dtype: needs fp32-like range
    Exp = mybir.ActivationFunctionType.Exp

    nc = bacc.Bacc("TRN2", target_bir_lowering=False, debug=False)

    xT_d = nc.dram_tensor("xT", [H, N], FR, kind="ExternalInput").ap()
    qT_d = nc.dram_tensor("qT", [H, NQ], FR, kind="ExternalInput").ap()
    hd_d = nc.dram_tensor("hd", [2, P, HDW], FR, kind="ExternalInput").ap()
    v_d = nc.dram_tensor("v", [N, H + 1], AVT, kind="ExternalInput").ap()
    att_d = nc.dram_tensor("att", [NQ, H], AVT, kind="ExternalOutput").ap()
    warm_d = nc.dram_tensor("warm", [P, 4], f32, kind="ExternalOutput").ap()

    with tile.TileContext(nc) as tc:
        with tc.tile_pool(name="io", bufs=1) as io, \
             tc.tile_pool(name="kqv", bufs=1) as kqv, \
             tc.tile_pool(name="expp", bufs=JC + 11) as expp, \
             tc.tile_pool(name="op", bufs=4) as op, \
             tc.tile_pool(name="psmm", bufs=3, space="PSUM") as psmm, \
             tc.tile_pool(name="psatt", bufs=5, space="PSUM") as psatt:

            xt = [io.tile([P, N], FR, tag=f"xt{h}", name=f"xt{h}") for h in range(2)]
            qt = [io.tile([P, NQ], FR, tag=f"qt{h}", name=f"qt{h}") for h in range(2)]
            hd = [io.tile([P, HDW], FR, tag=f"hd{h}", name=f"hd{h}") for h in range(2)]
            vt = [kqv.tile([P, H + 1], AVT, tag=f"v{j}", name=f"v{j}") for j in range(JC)]
            # per-key exp bias w_j rides the h=0 head blob as bit-cast
            # fp16 columns -- no separate (slow, 128B-row) wsc chain and
            # no extra sem before the first exp
            wsc = hd[0][:, HDX + IBLK:HDW].bitcast(f32)

            def ld_h(eng, sb_tiles, dr, h, cols):
                hs = slice(h * P, (h + 1) * P)
                eng.dma_start(sb_tiles[h][:, cols], dr[hs, cols])

            def ld_v(eng, j):
                eng.dma_start(vt[j][:], v_d[j * P:(j + 1) * P, :])

            # keys chunk stationary: the first 4 chunks live in the head
            # blob (their xt big-tile columns are never loaded).  Blob
            # layout: [xt jc0 | qt block0 | xt jc1..3 | wsc] so the first
            # chain (0:640) is exactly the first matmul pair's operands.
            def xstat(oc, jc):
                if jc == 0:
                    return hd[oc][:, 0:P]
                if jc < HDX // P:
                    return hd[oc][:, P + IBLK + (jc - 1) * P:
                                   P + IBLK + jc * P]
                return xt[oc][:, jc * P:(jc + 1) * P]

            # query block moving operand: block 0 lives in the head blob
            def qmov(oc, babs):
                if babs == 0:
                    return hd[oc][:, P:P + IBLK]
                return qt[oc][:, babs * IBLK:(babs + 1) * IBLK]

            # scalar ring: h=1 head blob (two chains: the first is just
            # the first matmul pair's operands, landing ~0.6us earlier)
            nc.scalar.dma_start(hd[1][:, 0:P + IBLK], hd_d[1, :, 0:P + IBLK])
            nc.scalar.dma_start(hd[1][:, P + IBLK:HDW], hd_d[1, :, P + IBLK:HDW])
            nc.scalar.dma_start(vt[1][:], v_d[P:2 * P, :])
            nc.scalar.dma_start(qt[1][:, 512:1024], qT_d[P:2 * P, 512:1024])

            # gpsimd SWDGE ring (otherwise idle): v tiles needed at
            # 17-22us; SWDGE's ~2.2us/chain serial emission delivers them
            # by ~16us, taking ~1.8us of load off the sync ring's
            # jitter-sensitive early window.
            for j in range(4, 8):
                ld_v(nc.gpsimd, j)

            # ---- PE warm-up on the const-AP region: it is memset during
            # the Bass preamble and fenced by the init all-engine barrier,
            # so the junk matmuls (and the dummy exp that pulls in the
            # ~2.7us ACT_TABLE_LOAD) start right after the barrier
            # (~7.0us) with no cross-engine sem hop.
            import concourse.mybir as _mybir
            cb = nc.const_aps.tensor(1.0, (P, P), _mybir.dt.bfloat16)
            cm = nc.const_aps.tensor(1.0, (P, 64), _mybir.dt.bfloat16)
            cw = nc.const_aps.tensor(0.0, (P, 2), f32)
            wex = op.tile([P, 2], f32, tag="wex", name="wex")
            nc.scalar.activation(wex[:], cw, Exp)

            # warmup matmuls: bridge until the head blob lands (~11.5us);
            # a PE gap before the first scores matmul would reset the HAM
            # activity window (costs ~2us of half-clock).
            wps = psmm.tile([P, 256], f32, tag="mm", name="wps")
            nw = nwarm + salt
            for r in range(nw):
                nc.tensor.matmul(wps[:, 0:64], cb, cm,
                                 start=(r == 0), stop=(r == nw - 1))
            wsb = op.tile([P, 4], f32, tag="wsb", name="wsb")
            nc.vector.tensor_copy(wsb[:, 0:2], wps[:, 0:2])
            nc.vector.tensor_copy(wsb[:, 2:4], wex[:])

            # sync ring: h=0 head blob, then wsc + v0 (small, they gate
            # the first exp/AV), then criticals and all bulk in
            # need-order (xt chains promoted ahead of same-time v tiles)
            nc.sync.dma_start(hd[0][:, 0:P + IBLK], hd_d[0, :, 0:P + IBLK])
            nc.sync.dma_start(hd[0][:, P + IBLK:HDW], hd_d[0, :, P + IBLK:HDW])
            ld_v(nc.sync, 0)
            ld_h(nc.sync, qt, qT_d, 0, slice(512, 1024))
            ld_h(nc.sync, xt, xT_d, 0, slice(512, 1024))
            ld_h(nc.sync, xt, xT_d, 1, slice(512, 1024))
            ld_v(nc.sync, 2)
            ld_v(nc.sync, 3)
            ld_h(nc.sync, xt, xT_d, 0, slice(1024, 2048))
            ld_h(nc.sync, xt, xT_d, 1, slice(1024, 2048))
            for j in range(8, 10):
                ld_v(nc.sync, j)
            ld_h(nc.sync, xt, xT_d, 0, slice(2048, 3072))
            ld_h(nc.sync, xt, xT_d, 1, slice(2048, 3072))
            for j in range(10, 18):
                ld_v(nc.sync, j)
            ld_h(nc.sync, xt, xT_d, 0, slice(3072, 4096))
            ld_h(nc.sync, xt, xT_d, 1, slice(3072, 4096))
            for j in range(18, 26):
                ld_v(nc.sync, j)
            ld_h(nc.sync, qt, qT_d, 0, slice(1024, 2048))
            ld_h(nc.sync, qt, qT_d, 1, slice(1024, 2048))
            for j in range(26, JC):
                ld_v(nc.sync, j)

            # warm-up flush (keeps junk matmuls + dummy exp alive through
            # DCE); last on the sync ring so it takes no early dispatch slot.
            nc.sync.dma_start(warm_d[:], wsb[:])

            # ---- attention block pairs ----
            # The scores matmuls for both blocks of a pair share each
            # key-chunk stationary load.  Block b0's AV runs inline per
            # key-chunk; block b1's exp(S) tiles are buffered in SBUF and
            # consumed in a second AV sweep (PSUM can only hold one
            # block's accumulators plus the rotating scores tiles).
            def normalize_one(att_tile, blk, ic):
                rec = op.tile([P, 1], f32, tag="rec", name="rec")
                nc.vector.reciprocal(rec[:], att_tile[:, H:H + 1])
                ao = op.tile([P, H], AVT, tag="ao", name="ao")
                nc.vector.tensor_scalar_mul(ao[:], att_tile[:, 0:H], rec[:])
                r0 = blk * IBLK + ic * P
                nc.sync.dma_start(att_d[r0:r0 + P, :], ao[:])

            def emit_scores(pair, jc, b, scs):
                babs = 2 * pair + b
                for oc in range(2):
                    nc.tensor.matmul(scs[:], xstat(oc, jc), qmov(oc, babs),
                                     start=(oc == 0), stop=(oc == 1))

            def emit_exp(jc, b, scs, split=False):
                ex = expp.tile([P, IBLK], AVT, tag="ex", name=f"ex{b}")
                if split:
                    # split the pair's first exp so its low half (feeding
                    # AV ic=0,1) is ready sooner (ACT has ~290ns/op fixed
                    # overhead, so only a 2-way split pays)
                    nc.scalar.activation(ex[:, 0:256], scs[:, 0:256], Exp,
                                         bias=wsc[:, jc:jc + 1])
                    nc.scalar.activation(ex[:, 256:IBLK], scs[:, 256:IBLK],
                                         Exp, bias=wsc[:, jc:jc + 1])
                else:
                    nc.scalar.activation(ex[:], scs[:], Exp,
                                         bias=wsc[:, jc:jc + 1])
                return ex

            def emit_av(att_ps, exs0, jc, vslice=None, ps_narrow=None):
                for ic in range(ICH):
                    ics = slice(ic * P, (ic + 1) * P)
                    nc.tensor.matmul(att_ps[ic][:], exs0[jc][:, ics],
                                     vt[jc][:],
                                     start=(jc == 0), stop=(jc == JC - 1))

            def emit_pair1_head():
                # pre-emit pair 1's jc0 (both blocks) + jc1-b0 scores and
                # exps; the PE runs them just before pair 0's final AV
                # sweep and the ACT ops complete during it, so pair 1's
                # first AV matmuls fire with no refill stall.
                ex0, ex1 = [], []
                s00 = psmm.tile([P, IBLK], f32, tag="mm", name="p1s00")
                emit_scores(1, 0, 0, s00)
                ex0.append(emit_exp(0, 0, s00))
                s01 = psmm.tile([P, IBLK], f32, tag="mm", name="p1s01")
                emit_scores(1, 0, 1, s01)
                ex1.append(emit_exp(0, 1, s01))
                s10 = psmm.tile([P, IBLK], f32, tag="mm", name="p1s10")
                emit_scores(1, 1, 0, s10)
                ex0.append(emit_exp(1, 0, s10))
                return ex0, ex1

            handoff = None
            for pair in range(NPAIR):
                att_ps = [psatt.tile([P, H + 1], f32, tag="att", name="attps")
                          for _ in range(ICH)]
                exs = [[], []]

                if pair == 0:
                    # data-arrival region: ALL of b0's first 4 key chunks
                    # (served by the head blob + early chains) run before
                    # any b1 work, giving the second-wave qt chains ~1.2us
                    # of jitter slack; AVs are interleaved so the exp->AV
                    # latency is covered by scores work.
                    sb0 = []
                    for jc in range(4):
                        s = psmm.tile([P, IBLK], f32, tag="mm",
                                      name=f"a{jc}")
                        emit_scores(0, jc, 0, s)
                        # no exp split: with wsc riding the head blob the
                        # exp starts early anyway, and a single op frees
                        # the scores PSUM slot ~0.4us sooner (the S03
                        # slot-wait seen in traces)
                        exs[0].append(emit_exp(jc, 0, s))
                        sb0.append(s)
                        if jc == 2:
                            emit_av(att_ps, exs[0], 0)
                        elif jc == 3:
                            emit_av(att_ps, exs[0], 1)
                    for jc in range(4):
                        s = psmm.tile([P, IBLK], f32, tag="mm",
                                      name=f"b{jc}")
                        emit_scores(0, jc, 1, s)
                        exs[1].append(emit_exp(jc, 1, s))
                        if jc == 1:
                            emit_av(att_ps, exs[0], 2)
                        elif jc == 2:
                            emit_av(att_ps, exs[0], 3)
                    jc_start = 4
                elif handoff is not None:
                    # pair 1's first scores+exps were pre-emitted into
                    # pair 0's final AV sweep (covers the pair-boundary
                    # exp->AV refill stall)
                    exs[0].append(handoff[0][0])
                    exs[0].append(handoff[0][1])
                    exs[1].append(handoff[1][0])
                    emit_av(att_ps, exs[0], 0)
                    sc11 = psmm.tile([P, IBLK], f32, tag="mm", name="sc11b")
                    emit_scores(1, 1, 1, sc11)
                    exs[1].append(emit_exp(1, 1, sc11))
                    emit_av(att_ps, exs[0], 1)
                    jc_start = 2
                else:
                    jc_start = 0

                for jc in range(jc_start, JC):
                    scs = [psmm.tile([P, IBLK], f32, tag="mm", name=f"sc{b}")
                           for b in range(2)]
                    if pair == 0 and jc < 4:
                        # block-major: b0's operands land ~1us before b1's
                        for b in range(2):
                            emit_scores(pair, jc, b, scs[b])
                            exs[b].append(emit_exp(jc, b, scs[b]))
                    else:
                        for oc in range(2):
                            for b in range(2):
                                nc.tensor.matmul(scs[b][:], xstat(oc, jc),
                                                 qmov(oc, 2 * pair + b),
                                                 start=(oc == 0),
                                                 stop=(oc == 1))
                        for b in range(2):
                            exs[b].append(emit_exp(jc, b, scs[b],
                                                   split=(jc == 0)))
                    emit_av(att_ps, exs[0], jc)

                for ic in range(ICH):
                    normalize_one(att_ps[ic], 2 * pair, ic)
                last = (pair == NPAIR - 1)
                for ic in range(ICH):
                    ics = slice(ic * P, (ic + 1) * P)
                    if last and ic == ICH - 1:
                        # final accumulator: split by V columns across two
                        # PSUM banks so the high half's normalize + DMA-out
                        # overlaps the low half's AV sweep.
                        pa = psatt.tile([P, H - P + 1], f32, tag="att",
                                        name="attpa")
                        pb = psatt.tile([P, P], f32, tag="att", name="attpb")
                        for jc in range(JC):
                            nc.tensor.matmul(pa[:], exs[1][jc][:, ics],
                                             vt[jc][:, P:H + 1],
                                             start=(jc == 0),
                                             stop=(jc == JC - 1))
                        rec = op.tile([P, 1], f32, tag="rec", name="rec")
                        nc.vector.reciprocal(rec[:], pa[:, H - P:H - P + 1])
                        ah = op.tile([P, H - P], AVT, tag="ao", name="ah")
                        nc.vector.tensor_scalar_mul(ah[:], pa[:, 0:H - P],
                                                    rec[:])
                        r0 = (2 * pair + 1) * IBLK + ic * P
                        nc.scalar.dma_start(att_d[r0:r0 + P, P:H], ah[:])
                        for jc in range(JC):
                            nc.tensor.matmul(pb[:], exs[1][jc][:, ics],
                                             vt[jc][:, 0:P],
                                             start=(jc == 0),
                                             stop=(jc == JC - 1))
                        # final normalize + out split in half across BOTH
                        # rings: the two DIRECT2D dispatches run in
                        # parallel and each 16KB transfer is ~0.5us vs
                        # ~0.9us for one 32KB chain at end-of-NEFF
                        al0 = op.tile([P, 64], AVT, tag="ao", name="al0")
                        nc.vector.tensor_scalar_mul(al0[:], pb[:, 0:64],
                                                    rec[:])
                        nc.scalar.dma_start(att_d[r0:r0 + P, 0:64], al0[:])
                        al1 = op.tile([P, 64], AVT, tag="ao", name="al1")
                        nc.vector.tensor_scalar_mul(al1[:], pb[:, 64:P],
                                                    rec[:])
                        nc.sync.dma_start(att_d[r0:r0 + P, 64:P], al1[:])
                    else:
                        if pair == 0 and ic == ICH - 1:
                            handoff = emit_pair1_head()
                        pf = psatt.tile([P, H + 1], f32, tag="att",
                                        name="attpsb")
                        for jc in range(JC):
                            nc.tensor.matmul(pf[:], exs[1][jc][:, ics],
                                             vt[jc][:],
                                             start=(jc == 0),
                                             stop=(jc == JC - 1))
                        normalize_one(pf, 2 * pair + 1, ic)

    nc.compile()
    return nc


_NC_CACHE = {}


def _get_nc():
    if "nc" not in _NC_CACHE:
        _NC_CACHE["nc"] = build_nc()
    return _NC_CACHE["nc"]


def _make_in_maps(x, Wq, bq, Wk, bk, Wv):
    import ml_dtypes

    bf16 = ml_dtypes.bfloat16
    A = Wq.T.astype(np.float64) @ Wk.astype(np.float64)
    wkbq = Wk.T.astype(np.float64) @ bq.astype(np.float64)
    in_maps = []
    for b in range(B):
        xb = x[b].astype(np.float64)
        wsc_b = np.ascontiguousarray(
            (xb @ wkbq).astype(np.float32).reshape(JC, P).T)
        v_b = np.empty((N, H + 1), dtype=bf16)
        v_b[:, 0:H] = (xb @ Wv.T.astype(np.float64)).astype(bf16)
        v_b[:, H:] = np.ones((N, 1), dtype=bf16)
        v_b = np.ascontiguousarray(v_b)
        xT_b = np.ascontiguousarray(x[b].astype(np.float16).T)
        q_b = (xb @ A).astype(np.float16)
        for half in range(2):
            qT = np.ascontiguousarray(q_b[half * NQ:(half + 1) * NQ, :].T)
            hd_b = np.empty((2, P, HDW), dtype=np.float16)
            for h in range(2):
                hs = slice(h * P, (h + 1) * P)
                hd_b[h, :, 0:P] = xT_b[hs, 0:P]
                hd_b[h, :, P:P + IBLK] = qT[hs, 0:IBLK]
                hd_b[h, :, P + IBLK:HDX + IBLK] = xT_b[hs, P:HDX]
                hd_b[h, :, HDX + IBLK:HDW] = wsc_b.view(np.float16)
            in_maps.append({"xT": xT_b, "qT": qT, "hd": hd_b, "v": v_b})
    return in_maps


def _run(inputs, trace=False):
    from concourse.bass_utils import run_bass_kernel_spmd

    x = np.asarray(inputs["x"], dtype=np.float32)
    Wq = np.asarray(inputs["Wq"], dtype=np.float32)
    bq = np.asarray(inputs["bq"], dtype=np.float32)
    Wk = np.asarray(inputs["Wk"], dtype=np.float32)
    bk = np.asarray(inputs["bk"], dtype=np.float32)
    Wv = np.asarray(inputs["Wv"], dtype=np.float32)
    bv = np.asarray(inputs["bv"], dtype=np.float32)

    in_maps = _make_in_maps(x, Wq, bq, Wk, bk, Wv)
    # The device occasionally wedges on the first execution of a fresh
    # NEFF (NRT_EXEC_UNIT_UNRECOVERABLE) or silently corrupts an output
    # (NaN/garbage); a retry -- with a slightly perturbed program
    # (different walrus schedule) on exception -- recovers.
    last_exc = None
    out = None
    for attempt in range(4):
        try:
            nc = _get_nc() if attempt < 2 else build_nc(salt=attempt)
            res = run_bass_kernel_spmd(nc, in_maps, list(range(8)), trace=trace)
        except Exception as e:  # noqa: BLE001
            last_exc = e
            import os as _os
            import time as _time
            _os.environ["NEURON_RT_RESET_CORES"] = "1"
            _time.sleep(3)
            continue
        out = np.empty((B, N, H), dtype=np.float32)
        for c in range(8):
            b, half = c // 2, c % 2
            out[b, half * NQ:(half + 1) * NQ, :] = \
                res.results[c]["att"].astype(np.float32) + bv
        if np.isfinite(out).all() and np.abs(out).max() < 1e3:
            return out, res
    if out is None:
        raise last_exc
    return out, res


def kernel(**inputs) -> np.ndarray:
    out, _ = _run(inputs, trace=False)
    return out
